# revision 15
# baseline (speedup 1.0000x reference)
"""Trainium2 Bass kernel for nn_PairwiseConv (gnn_message_passing).

Reference computation, for each edge e=(i,j) of a sparse adjacency:
    pair[b,o,e] = sum_c W[o,c,0]*x[b,c,i] + W[o,c,1]*x[b,c,j] + bias[o]
    y[b,o,n]    = (sum_{e: i_e=n} pair[b,o,e]) / max(deg_j[n],1)
    y[b,127,n]  = deg_j[n]            (counts channel)
where deg_j[n] = #{e: j_e = n}.

Algebraic reformulation used here (exact):
    y[b,o,n] = (deg_i[n]*(W0x[b,o,n] + bias[o]) + S[b,o,n]) / max(deg_j[n],1)
    S[b,o,n] = sum_m z[b,o,m] * AT[m,n],   z = W1^T x   (plus an all-ones
               row o=127 so that S[b,127,n] = deg_i[n])
    AT[m,n]  = #{e: j_e = m, i_e = n}  (edge-count matrix)
so the irregular gather/scatter becomes one dense [128,4096]x[4096,512]
matmul per (batch, node-slice) against the on-device-built count matrix.

Sharding: 8 cores = 8 slices of 512 output nodes; each core computes all 4
batches for its slice. AT[:, slice] is built on device from host-packed
per-partition (index,count) tables via GPSIMD local_scatter (32 tiles of
[128 rows, 512 cols], one per 128-row chunk of the source-node axis).
deg_j is built the same way into a [128,512] count matrix C (edges spread
round-robin over the 128 partitions) and reduced with an all-ones matmul,
which also broadcasts deg_j to all 128 partitions.

Host-side work is limited to formatting: slicing/deduplicating edge lists
into padded scatter tables, rotating x so every core sees its slice at
column 0 (keeps the SPMD program identical across cores), and
concatenating the 8 output tiles.
"""

import os

import numpy as np
import ml_dtypes

import concourse.bass as bass
import concourse.mybir as mybir
import concourse.tile as tile
from concourse import bacc
from concourse.bass_utils import run_bass_kernel_spmd

B = 4
C = 128  # in channels
O = 128  # out channels incl. counts row (127 real + ones row)
N = 4096
SLICE = 512  # output nodes per core
NCORES = 8
MC = N // 128  # 32 source-node chunks
F32 = mybir.dt.float32
BF16 = mybir.dt.bfloat16
I16 = mybir.dt.int16
BF16_NP = ml_dtypes.bfloat16


def _pack_tables(rows, cols, nrows, ncols, ni=None):
    """Group (row, col) pairs by partition p=row%128 (and chunk row//128),
    dedup, and pack into [128, nchunk*NI] int16 index / bf16 count tables.

    rows in [0, nrows), cols in [0, ncols). Returns (idx, val, NI).
    """
    nchunk = nrows // 128
    key = rows * ncols + cols
    uniq, counts = np.unique(key, return_counts=True)
    ur = uniq // ncols
    uc = uniq % ncols
    chunk = ur // 128
    p = ur % 128
    # sort by (chunk, p) to get per-(chunk,p) runs
    order = np.lexsort((uc, p, chunk))
    chunk, p, uc, counts = chunk[order], p[order], uc[order], counts[order]
    gid = chunk * 128 + p
    # per (chunk,p) counts
    percell = np.bincount(gid, minlength=nchunk * 128)
    ni = ni if ni is not None else int(percell.max())
    ni += ni % 2  # even
    ni = max(ni, 2)
    idx = np.full((nchunk * 128, ni), -1, np.int16)
    val = np.zeros((nchunk * 128, ni), BF16_NP)
    pos = np.arange(len(gid)) - np.concatenate(([0], np.cumsum(percell)))[gid]
    idx[gid, pos] = uc.astype(np.int16)
    val[gid, pos] = counts.astype(BF16_NP)
    # [nchunk*128, ni] -> [128, nchunk*ni]
    idx = idx.reshape(nchunk, 128, ni).transpose(1, 0, 2).reshape(128, nchunk * ni)
    val = val.reshape(nchunk, 128, ni).transpose(1, 0, 2).reshape(128, nchunk * ni)
    return np.ascontiguousarray(idx), np.ascontiguousarray(val), ni


def prep_inputs(x, W, b, idx_i, idx_j):
    """Returns (in_maps, NI_A, NI_C): per-core input dicts + table widths."""
    x = np.ascontiguousarray(np.asarray(x, np.float32))
    W = np.asarray(W, np.float32)
    bias = np.asarray(b, np.float32)
    ii = np.asarray(idx_i).astype(np.int64)
    jj = np.asarray(idx_j).astype(np.int64)

    # weights: lhsT layouts [K=c, M=o], padded to 128 with a zero column
    W0T = np.zeros((128, 128), BF16_NP)
    W0T[:, :127] = W[:, :, 0].T.astype(BF16_NP)
    W1T = np.zeros((128, 128), BF16_NP)
    W1T[:, :127] = W[:, :, 1].T.astype(BF16_NP)
    bcol = np.zeros((128, 1), np.float32)
    bcol[:127, 0] = bias

    # first pass: compute per-core tables, track global max widths
    perc = []
    for s in range(NCORES):
        base = s * SLICE
        # AT build: edges with destination i in slice; row = rotated source
        sel = (ii >= base) & (ii < base + SLICE)
        m_rot = (jj[sel] - base) % N
        icol = ii[sel] - base
        # pack chunk-pairs: row' in [0, N/2), col' in [0, 1024)
        a_rows = (m_rot // 256) * 128 + (m_rot % 128)
        a_cols = icol + SLICE * ((m_rot // 128) % 2)
        # deg_j build: edges with j in slice, spread over partitions
        selj = (jj >= base) & (jj < base + SLICE)
        nj = int(selj.sum())
        c_rows = np.arange(nj, dtype=np.int64) % 128
        c_cols = jj[selj] - base
        # deg_i build: the i-filtered edge set, spread over partitions
        ni_ = int(sel.sum())
        d_rows = np.arange(ni_, dtype=np.int64) % 128
        d_cols = icol
        perc.append((a_rows, a_cols, c_rows, c_cols, d_rows, d_cols))

    # uniform NI across cores (SPMD program shapes must match)
    ni_a = ni_c = 0
    for a_rows, a_cols, c_rows, c_cols, d_rows, d_cols in perc:
        _, _, na = _pack_tables(a_rows, a_cols, N // 2, 2 * SLICE)
        _, _, nc_ = _pack_tables(c_rows, c_cols, 128, SLICE)
        _, _, nd = _pack_tables(d_rows, d_cols, 128, SLICE)
        ni_a, ni_c = max(ni_a, na), max(ni_c, max(nc_, nd))

    in_maps = []
    for s in range(NCORES):
        a_rows, a_cols, c_rows, c_cols, d_rows, d_cols = perc[s]
        idxA, valA, _ = _pack_tables(a_rows, a_cols, N // 2, 2 * SLICE, ni=ni_a)
        idxC, valC, _ = _pack_tables(c_rows, c_cols, 128, SLICE, ni=ni_c)
        idxD, valD, _ = _pack_tables(d_rows, d_cols, 128, SLICE, ni=ni_c)
        m = {
            "W0T": W0T,
            "W1T": W1T,
            "bcol": bcol,
            "idxA": idxA,
            "valA": valA,
            "idxC": np.ascontiguousarray(np.concatenate([idxC, idxD], axis=1)),
            "valC": np.ascontiguousarray(np.concatenate([valC, valD], axis=1)),
        }
        for bi in range(B):
            m[f"x{bi}"] = np.ascontiguousarray(
                np.roll(x[bi], -s * SLICE, axis=1).astype(BF16_NP))
        in_maps.append(m)
    return in_maps, ni_a, ni_c


def build_program(ni_a, ni_c):
    nc = bacc.Bacc("TRN2", target_bir_lowering=False, debug=False, num_devices=NCORES)

    xs = [nc.dram_tensor(f"x{bi}", [C, N], BF16, kind="ExternalInput") for bi in range(B)]
    W0T = nc.dram_tensor("W0T", [128, 128], BF16, kind="ExternalInput")
    W1T = nc.dram_tensor("W1T", [128, 128], BF16, kind="ExternalInput")
    bcol = nc.dram_tensor("bcol", [128, 1], F32, kind="ExternalInput")
    idxA = nc.dram_tensor("idxA", [128, (MC // 2) * ni_a], I16, kind="ExternalInput")
    valA = nc.dram_tensor("valA", [128, (MC // 2) * ni_a], BF16, kind="ExternalInput")
    idxC = nc.dram_tensor("idxC", [128, 2 * ni_c], I16, kind="ExternalInput")
    valC = nc.dram_tensor("valC", [128, 2 * ni_c], BF16, kind="ExternalInput")
    youts = [nc.dram_tensor(f"y{bi}", [O, SLICE], F32, kind="ExternalOutput")
             for bi in range(B)]

    with tile.TileContext(nc) as tc:
        with (
            tc.tile_pool(name="const", bufs=1) as constp,
            tc.tile_pool(name="scat", bufs=1) as scatp,
            tc.tile_pool(name="at", bufs=1) as atp,
            tc.tile_pool(name="xp", bufs=1) as xp,
            tc.tile_pool(name="zt", bufs=1) as ztp,
            tc.tile_pool(name="work", bufs=1) as workp,
            tc.tile_pool(name="small", bufs=4) as smallp,
            tc.tile_pool(name="ps_g", bufs=1, space="PSUM") as ps_g,
            tc.tile_pool(name="ps_misc", bufs=2, space="PSUM") as ps_misc,
            tc.tile_pool(name="ps_deg", bufs=1, space="PSUM") as ps_deg_p,
            tc.tile_pool(name="ps_di", bufs=1, space="PSUM") as ps_di_p,
        ):
            # ---- loads: x on sync+scalar HWDGE queues, tables on gpsimd ----
            w1t = constp.tile([128, 128], BF16)
            nc.sync.dma_start(w1t[:], W1T[:])
            half = N // 2
            xbs, xts = [], []
            for bi in range(B):
                xb = xp.tile([C, N], BF16, tag=f"xb{bi}", name=f"xb{bi}")
                eng = nc.sync if bi < 2 else nc.scalar
                eng.dma_start(xb[:, :half], xs[bi][:, :half])
                eng.dma_start(xb[:, half:], xs[bi][:, half:])
                xbs.append(xb)
            iC = scatp.tile([128, 2 * ni_c], I16)
            nc.gpsimd.dma_start(iC[:], idxC[:])
            vC = scatp.tile([128, 2 * ni_c], BF16)
            nc.gpsimd.dma_start(vC[:], valC[:])
            iA = scatp.tile([128, (MC // 2) * ni_a], I16)
            nc.gpsimd.dma_start(iA[:], idxA[:])
            vA = scatp.tile([128, (MC // 2) * ni_a], BF16)
            nc.gpsimd.dma_start(vA[:], valA[:])
            w0t = constp.tile([128, 128], BF16)
            nc.scalar.dma_start(w0t[:], W0T[:])
            bc = constp.tile([128, 1], F32)
            nc.scalar.dma_start(bc[:], bcol[:])
            ones128 = constp.tile([128, 128], BF16)
            nc.vector.memset(ones128[:], 1.0)

            # xT via DMA transpose (SBUF->SBUF, xbar): [c, m] -> [m%128, m//128, c]
            for bi in range(B):
                xt = ztp.tile([128, N], BF16, tag=f"xt{bi}", name=f"xt{bi}")
                xt3 = xt[:].rearrange("p (a b) -> p a b", b=128)
                eng = nc.sync if bi < 2 else nc.scalar
                eng.dma_start(xt3[:, :MC // 2, :], xbs[bi][:, :half], transpose=True)
                eng.dma_start(xt3[:, MC // 2:, :], xbs[bi][:, half:], transpose=True)
                xts.append(xt)

            # ---- count-matrix scatters (GPSIMD): deg_j, deg_i, then AT ----
            cC = constp.tile([128, SLICE], BF16)
            nc.gpsimd.local_scatter(
                out_ap=cC[:], data_ap=vC[:, :ni_c], idxs_ap=iC[:, :ni_c],
                channels=128, num_elems=SLICE, num_idxs=ni_c,
            )
            cI = constp.tile([128, SLICE], BF16)
            nc.gpsimd.local_scatter(
                out_ap=cI[:], data_ap=vC[:, ni_c:], idxs_ap=iC[:, ni_c:],
                channels=128, num_elems=SLICE, num_idxs=ni_c,
            )
            at = atp.tile([128, MC * SLICE], BF16)
            for k in range(MC // 2):
                nc.gpsimd.local_scatter(
                    out_ap=at[:, k * 1024:(k + 1) * 1024],
                    data_ap=vA[:, k * ni_a:(k + 1) * ni_a],
                    idxs_ap=iA[:, k * ni_a:(k + 1) * ni_a],
                    channels=128, num_elems=1024, num_idxs=ni_a,
                )

            # ---- degree reductions + u_b prep ----
            ps_deg = ps_deg_p.tile([128, SLICE], F32)
            nc.tensor.matmul(ps_deg[:], ones128[:], cC[:], start=True, stop=True)
            ps_di = ps_di_p.tile([128, SLICE], F32)
            nc.tensor.matmul(ps_di[:], ones128[:], cI[:], start=True, stop=True)
            degj_raw = smallp.tile([1, SLICE], F32)
            nc.scalar.copy(degj_raw[:], ps_deg[0:1, :])
            rmax = workp.tile([128, SLICE], F32)
            nc.vector.tensor_scalar_max(rmax[:], ps_deg[:], 1.0)
            recip = workp.tile([128, SLICE], F32)
            nc.vector.reciprocal(recip[:], rmax[:])

            t1s = []
            for bi in range(B):
                ps_u = ps_misc.tile([128, 512], F32, tag="pm", name=f"ps_u{bi}")
                nc.tensor.matmul(ps_u[:], w0t[:], xbs[bi][:, :SLICE],
                                 start=True, stop=True)
                ub = smallp.tile([128, SLICE], F32, tag="ub", name=f"ub{bi}")
                nc.vector.tensor_scalar_add(ub[:], ps_u[:], bc[:, :1])
                t1 = smallp.tile([128, SLICE], F32, tag=f"t1{bi}", name=f"t1{bi}")
                nc.vector.tensor_mul(t1[:], ub[:], ps_di[:])
                t1s.append(t1)

            # ---- big matmuls: G_b = x_b @ AT, chunk-pair wavefront with
            #      batches 2,3 lagging so they start after their xT lands ----
            ps_Gs = [ps_g.tile([128, SLICE], F32, tag=f"pg{bi}", name=f"ps_G{bi}")
                     for bi in range(B)]
            started = [False] * B

            def emit_pair(bi, k):
                for mc in (2 * k, 2 * k + 1):
                    nc.tensor.matmul(
                        ps_Gs[bi][:],
                        xts[bi][:, mc * 128:(mc + 1) * 128],
                        at[:, mc * SLICE:(mc + 1) * SLICE],
                        start=(not started[bi]) if mc == 2 * k else False,
                        stop=(mc == MC - 1),
                        skip_group_check=True,
                    )
                    started[bi] = True

            LAG = 6
            NP = MC // 2  # 16 chunk-pairs
            for k in range(NP):
                emit_pair(0, k)
                emit_pair(1, k)
                if k >= LAG:
                    emit_pair(2, k - LAG)
                    emit_pair(3, k - LAG)
            for k in range(NP - LAG, NP):
                emit_pair(2, k)
                emit_pair(3, k)

            # ---- epilogue per batch: S = W1^T G, combine, out ----
            for bi in range(B):
                g_sb = smallp.tile([128, SLICE], BF16, tag="gsb", name=f"g_sb{bi}")
                nc.vector.tensor_copy(g_sb[:], ps_Gs[bi][:])
                ps_S = ps_misc.tile([128, SLICE], F32, tag="pm", name=f"ps_S{bi}")
                nc.tensor.matmul(ps_S[:], w1t[:], g_sb[:], start=True, stop=True)
                t2 = smallp.tile([128, SLICE], F32, tag="t2", name=f"t2{bi}")
                nc.vector.tensor_add(t2[:], t1s[bi][:], ps_S[:])
                ost = workp.tile([O, SLICE], F32, tag=f"ost{bi}", name=f"ost{bi}")
                nc.vector.tensor_mul(ost[:], t2[:], recip[:])
                nc.sync.dma_start(ost[127:128, :], degj_raw[:])
                nc.sync.dma_start(youts[bi][:], ost[:])

    nc.compile()
    return nc


def kernel(x, W, b, idx_i, idx_j):
    in_maps, ni_a, ni_c = prep_inputs(x, W, b, idx_i, idx_j)
    nc = build_program(ni_a, ni_c)
    res = run_bass_kernel_spmd(nc, in_maps, list(range(NCORES)))
    y = np.empty((B, O, N), np.float32)
    for s in range(NCORES):
        for bi in range(B):
            y[bi, :, s * SLICE:(s + 1) * SLICE] = res.results[s][f"y{bi}"]
    return y


if __name__ == "__main__":
    rng = np.random.default_rng(0)
    x = rng.standard_normal((B, C, N), np.float32)
    W = rng.standard_normal((127, C, 2), np.float32) * 0.05
    b = rng.standard_normal((127,), np.float32) * 0.05
    idx_i = rng.integers(0, N, 131072)
    idx_j = rng.integers(0, N, 131072)
    y = kernel(x, W, b, idx_i, idx_j)
    print("ok", y.shape, float(np.abs(y).mean()))


# revision 16
# speedup vs baseline: 1.1062x; 1.1062x over previous
"""Trainium2 Bass kernel for nn_PairwiseConv (gnn_message_passing).

Reference computation, for each edge e=(i,j) of a sparse adjacency:
    pair[b,o,e] = sum_c W[o,c,0]*x[b,c,i] + W[o,c,1]*x[b,c,j] + bias[o]
    y[b,o,n]    = (sum_{e: i_e=n} pair[b,o,e]) / max(deg_j[n],1)
    y[b,127,n]  = deg_j[n]            (counts channel)
where deg_j[n] = #{e: j_e = n}.

Algebraic reformulation used here (exact):
    y[b,o,n] = (deg_i[n]*(W0x[b,o,n] + bias[o]) + S[b,o,n]) / max(deg_j[n],1)
    S[b,o,n] = sum_m z[b,o,m] * AT[m,n],   z = W1^T x   (plus an all-ones
               row o=127 so that S[b,127,n] = deg_i[n])
    AT[m,n]  = #{e: j_e = m, i_e = n}  (edge-count matrix)
so the irregular gather/scatter becomes one dense [128,4096]x[4096,512]
matmul per (batch, node-slice) against the on-device-built count matrix.

Sharding: 8 cores = 8 slices of 512 output nodes; each core computes all 4
batches for its slice. AT[:, slice] is built on device from host-packed
per-partition (index,count) tables via GPSIMD local_scatter (32 tiles of
[128 rows, 512 cols], one per 128-row chunk of the source-node axis).
deg_j is built the same way into a [128,512] count matrix C (edges spread
round-robin over the 128 partitions) and reduced with an all-ones matmul,
which also broadcasts deg_j to all 128 partitions.

Host-side work is limited to formatting: slicing/deduplicating edge lists
into padded scatter tables, rotating x so every core sees its slice at
column 0 (keeps the SPMD program identical across cores), and
concatenating the 8 output tiles.
"""

import os

import numpy as np
import ml_dtypes

import concourse.bass as bass
import concourse.mybir as mybir
import concourse.tile as tile
from concourse import bacc
from concourse.bass_utils import run_bass_kernel_spmd

B = 4
C = 128  # in channels
O = 128  # out channels incl. counts row (127 real + ones row)
N = 4096
SLICE = 512  # output nodes per core
NCORES = 8
MC = N // 128  # 32 source-node chunks
F32 = mybir.dt.float32
BF16 = mybir.dt.bfloat16
I16 = mybir.dt.int16
BF16_NP = ml_dtypes.bfloat16


def _pack_tables(rows, cols, nrows, ncols, ni=None):
    """Group (row, col) pairs by partition p=row%128 (and chunk row//128),
    dedup, and pack into [128, nchunk*NI] int16 index / bf16 count tables.

    rows in [0, nrows), cols in [0, ncols). Returns (idx, val, NI).
    """
    nchunk = nrows // 128
    key = rows * ncols + cols
    uniq, counts = np.unique(key, return_counts=True)
    ur = uniq // ncols
    uc = uniq % ncols
    chunk = ur // 128
    p = ur % 128
    # sort by (chunk, p) to get per-(chunk,p) runs
    order = np.lexsort((uc, p, chunk))
    chunk, p, uc, counts = chunk[order], p[order], uc[order], counts[order]
    gid = chunk * 128 + p
    # per (chunk,p) counts
    percell = np.bincount(gid, minlength=nchunk * 128)
    ni = ni if ni is not None else int(percell.max())
    ni += ni % 2  # even
    ni = max(ni, 2)
    idx = np.full((nchunk * 128, ni), -1, np.int16)
    val = np.zeros((nchunk * 128, ni), BF16_NP)
    pos = np.arange(len(gid)) - np.concatenate(([0], np.cumsum(percell)))[gid]
    idx[gid, pos] = uc.astype(np.int16)
    val[gid, pos] = counts.astype(BF16_NP)
    # [nchunk*128, ni] -> [128, nchunk*ni]
    idx = idx.reshape(nchunk, 128, ni).transpose(1, 0, 2).reshape(128, nchunk * ni)
    val = val.reshape(nchunk, 128, ni).transpose(1, 0, 2).reshape(128, nchunk * ni)
    return np.ascontiguousarray(idx), np.ascontiguousarray(val), ni


def prep_inputs(x, W, b, idx_i, idx_j):
    """Returns (in_maps, NI_A, NI_C): per-core input dicts + table widths."""
    x = np.ascontiguousarray(np.asarray(x, np.float32))
    W = np.asarray(W, np.float32)
    bias = np.asarray(b, np.float32)
    ii = np.asarray(idx_i).astype(np.int64)
    jj = np.asarray(idx_j).astype(np.int64)

    # weights: lhsT layouts [K=c, M=o], padded to 128 with a zero column
    W0T = np.zeros((128, 128), BF16_NP)
    W0T[:, :127] = W[:, :, 0].T.astype(BF16_NP)
    W1T = np.zeros((128, 128), BF16_NP)
    W1T[:, :127] = W[:, :, 1].T.astype(BF16_NP)
    bcol = np.zeros((128, 1), np.float32)
    bcol[:127, 0] = bias

    # first pass: compute per-core tables, track global max widths
    perc = []
    for s in range(NCORES):
        base = s * SLICE
        # AT build: edges with destination i in slice; row = rotated source
        sel = (ii >= base) & (ii < base + SLICE)
        m_rot = (jj[sel] - base) % N
        icol = ii[sel] - base
        # pack chunk-pairs: row' in [0, N/2), col' in [0, 1024)
        a_rows = (m_rot // 256) * 128 + (m_rot % 128)
        a_cols = icol + SLICE * ((m_rot // 128) % 2)
        # deg_j build: edges with j in slice, spread over partitions
        selj = (jj >= base) & (jj < base + SLICE)
        nj = int(selj.sum())
        c_rows = np.arange(nj, dtype=np.int64) % 128
        c_cols = jj[selj] - base
        # deg_i build: the i-filtered edge set, spread over partitions
        ni_ = int(sel.sum())
        d_rows = np.arange(ni_, dtype=np.int64) % 128
        d_cols = icol
        perc.append((a_rows, a_cols, c_rows, c_cols, d_rows, d_cols))

    # uniform NI across cores (SPMD program shapes must match)
    ni_a = ni_c = 0
    for a_rows, a_cols, c_rows, c_cols, d_rows, d_cols in perc:
        _, _, na = _pack_tables(a_rows, a_cols, N // 2, 2 * SLICE)
        _, _, nc_ = _pack_tables(c_rows, c_cols, 128, SLICE)
        _, _, nd = _pack_tables(d_rows, d_cols, 128, SLICE)
        ni_a, ni_c = max(ni_a, na), max(ni_c, max(nc_, nd))

    in_maps = []
    for s in range(NCORES):
        a_rows, a_cols, c_rows, c_cols, d_rows, d_cols = perc[s]
        idxA, valA, _ = _pack_tables(a_rows, a_cols, N // 2, 2 * SLICE, ni=ni_a)
        idxC, valC, _ = _pack_tables(c_rows, c_cols, 128, SLICE, ni=ni_c)
        idxD, valD, _ = _pack_tables(d_rows, d_cols, 128, SLICE, ni=ni_c)
        m = {
            "W0T": W0T,
            "W1T": W1T,
            "bcol": bcol,
            "idxA": idxA,
            "valA": valA,
            "idxC": np.ascontiguousarray(np.concatenate([idxC, idxD], axis=1)),
            "valC": np.ascontiguousarray(np.concatenate([valC, valD], axis=1)),
        }
        for bi in range(B):
            m[f"x{bi}"] = np.ascontiguousarray(
                np.roll(x[bi], -s * SLICE, axis=1).astype(BF16_NP))
        in_maps.append(m)
    return in_maps, ni_a, ni_c


def build_program(ni_a, ni_c):
    nc = bacc.Bacc("TRN2", target_bir_lowering=False, debug=False, num_devices=NCORES)

    xs = [nc.dram_tensor(f"x{bi}", [C, N], BF16, kind="ExternalInput") for bi in range(B)]
    W0T = nc.dram_tensor("W0T", [128, 128], BF16, kind="ExternalInput")
    W1T = nc.dram_tensor("W1T", [128, 128], BF16, kind="ExternalInput")
    bcol = nc.dram_tensor("bcol", [128, 1], F32, kind="ExternalInput")
    idxA = nc.dram_tensor("idxA", [128, (MC // 2) * ni_a], I16, kind="ExternalInput")
    valA = nc.dram_tensor("valA", [128, (MC // 2) * ni_a], BF16, kind="ExternalInput")
    idxC = nc.dram_tensor("idxC", [128, 2 * ni_c], I16, kind="ExternalInput")
    valC = nc.dram_tensor("valC", [128, 2 * ni_c], BF16, kind="ExternalInput")
    youts = [nc.dram_tensor(f"y{bi}", [O, SLICE], F32, kind="ExternalOutput")
             for bi in range(B)]

    with tile.TileContext(nc) as tc:
        with (
            tc.tile_pool(name="const", bufs=1) as constp,
            tc.tile_pool(name="scat", bufs=1) as scatp,
            tc.tile_pool(name="at", bufs=1) as atp,
            tc.tile_pool(name="xp", bufs=1) as xp,
            tc.tile_pool(name="zt", bufs=1) as ztp,
            tc.tile_pool(name="work", bufs=1) as workp,
            tc.tile_pool(name="small", bufs=4) as smallp,
            tc.tile_pool(name="ps_zt", bufs=2, space="PSUM") as ps_zt,
            tc.tile_pool(name="ps_s", bufs=1, space="PSUM") as ps_s,
            tc.tile_pool(name="ps_deg", bufs=1, space="PSUM") as ps_deg_p,
            tc.tile_pool(name="ps_di", bufs=1, space="PSUM") as ps_di_p,
        ):
            # ---- loads: x on sync+scalar HWDGE queues, tables on gpsimd ----
            w1t = constp.tile([128, 128], BF16)
            nc.sync.dma_start(w1t[:], W1T[:])
            half = N // 2
            xbs = []
            for bi in range(B):
                xb = xp.tile([C, N], BF16, tag=f"xb{bi}", name=f"xb{bi}")
                eng = nc.sync if bi < 2 else nc.scalar
                if bi == 0:
                    q = N // 4
                    for qi in range(4):
                        eng.dma_start(xb[:, qi * q:(qi + 1) * q],
                                      xs[bi][:, qi * q:(qi + 1) * q])
                else:
                    eng.dma_start(xb[:, :half], xs[bi][:, :half])
                    eng.dma_start(xb[:, half:], xs[bi][:, half:])
                xbs.append(xb)
            iC = scatp.tile([128, 2 * ni_c], I16)
            nc.gpsimd.dma_start(iC[:], idxC[:])
            vC = scatp.tile([128, 2 * ni_c], BF16)
            nc.gpsimd.dma_start(vC[:], valC[:])
            iA = scatp.tile([128, (MC // 2) * ni_a], I16)
            nc.gpsimd.dma_start(iA[:], idxA[:])
            vA = scatp.tile([128, (MC // 2) * ni_a], BF16)
            nc.gpsimd.dma_start(vA[:], valA[:])
            w0t = constp.tile([128, 128], BF16)
            nc.scalar.dma_start(w0t[:], W0T[:])
            bc = constp.tile([128, 1], F32)
            nc.scalar.dma_start(bc[:], bcol[:])
            ones128 = constp.tile([128, 128], BF16)
            nc.vector.memset(ones128[:], 1.0)

            # ---- count-matrix scatters (GPSIMD): deg_j, deg_i, then AT ----
            cC = constp.tile([128, SLICE], BF16)
            nc.gpsimd.local_scatter(
                out_ap=cC[:], data_ap=vC[:, :ni_c], idxs_ap=iC[:, :ni_c],
                channels=128, num_elems=SLICE, num_idxs=ni_c,
            )
            cI = constp.tile([128, SLICE], BF16)
            nc.gpsimd.local_scatter(
                out_ap=cI[:], data_ap=vC[:, ni_c:], idxs_ap=iC[:, ni_c:],
                channels=128, num_elems=SLICE, num_idxs=ni_c,
            )
            at = atp.tile([128, MC * SLICE], BF16)
            for k in range(MC // 2):
                nc.gpsimd.local_scatter(
                    out_ap=at[:, k * 1024:(k + 1) * 1024],
                    data_ap=vA[:, k * ni_a:(k + 1) * ni_a],
                    idxs_ap=iA[:, k * ni_a:(k + 1) * ni_a],
                    channels=128, num_elems=1024, num_idxs=ni_a,
                )

            # ---- phase A: zT builds for all batches ----
            zts = []
            for bi in range(B):
                xb = xbs[bi]
                zt = ztp.tile([128, N], BF16, tag=f"zt{bi}", name=f"zt{bi}")
                for g in range(MC // 4):  # 8 psum groups of 4 chunks
                    pz = ps_zt.tile([128, 512], F32, tag="pz", name=f"pz{bi}_{g}")
                    for kk in range(4):
                        mc = g * 4 + kk
                        nc.tensor.matmul(
                            pz[:, kk * 128:(kk + 1) * 128],
                            xb[:, mc * 128:(mc + 1) * 128],
                            w1t[:],
                            start=True, stop=True,
                        )
                    if g % 2 == 0:
                        nc.vector.tensor_copy(zt[:, g * 512:(g + 1) * 512], pz[:])
                    else:
                        nc.scalar.copy(zt[:, g * 512:(g + 1) * 512], pz[:])
                zts.append(zt)

            # ---- degree reductions (batch-independent, all-ones matmuls) ----
            ps_deg = ps_deg_p.tile([128, SLICE], F32)
            nc.tensor.matmul(ps_deg[:], ones128[:], cC[:], start=True, stop=True)
            ps_di = ps_di_p.tile([128, SLICE], F32)
            nc.tensor.matmul(ps_di[:], ones128[:], cI[:], start=True, stop=True)
            degj_raw = smallp.tile([1, SLICE], F32)
            nc.scalar.copy(degj_raw[:], ps_deg[0:1, :])
            rmax = workp.tile([128, SLICE], F32)
            nc.vector.tensor_scalar_max(rmax[:], ps_deg[:], 1.0)
            recip = workp.tile([128, SLICE], F32)
            nc.vector.reciprocal(recip[:], rmax[:])

            # ---- u_b = W0^T x_b(slice); t1 = (u + bias) * deg_i ----
            t1s = []
            for bi in range(B):
                ps_u = ps_zt.tile([128, 512], F32, tag="pz", name=f"ps_u{bi}")
                nc.tensor.matmul(ps_u[:], w0t[:], xbs[bi][:, :SLICE],
                                 start=True, stop=True)
                ub = smallp.tile([128, SLICE], F32, tag="ub", name=f"ub{bi}")
                nc.vector.tensor_scalar_add(ub[:], ps_u[:], bc[:, :1])
                t1 = smallp.tile([128, SLICE], F32, tag=f"t1{bi}", name=f"t1{bi}")
                nc.vector.tensor_mul(t1[:], ub[:], ps_di[:])
                t1s.append(t1)

            # ---- phase B: big matmuls, chunk-major; tail batch-major ----
            TAIL = 4
            ps_Ss = [ps_s.tile([128, SLICE], F32, tag=f"ps{bi}", name=f"ps_S{bi}")
                     for bi in range(B)]
            for mc in range(MC - TAIL):
                for bi in range(B):
                    nc.tensor.matmul(
                        ps_Ss[bi][:],
                        zts[bi][:, mc * 128:(mc + 1) * 128],
                        at[:, mc * SLICE:(mc + 1) * SLICE],
                        start=(mc == 0), stop=False,
                        skip_group_check=True,
                    )
            for bi in range(B):
                for mc in range(MC - TAIL, MC):
                    nc.tensor.matmul(
                        ps_Ss[bi][:],
                        zts[bi][:, mc * 128:(mc + 1) * 128],
                        at[:, mc * SLICE:(mc + 1) * SLICE],
                        start=False, stop=(mc == MC - 1),
                        skip_group_check=True,
                    )
                t2 = smallp.tile([128, SLICE], F32, tag="t2", name=f"t2{bi}")
                nc.vector.tensor_add(t2[:], t1s[bi][:], ps_Ss[bi][:])
                ost = workp.tile([O, SLICE], F32, tag=f"ost{bi}", name=f"ost{bi}")
                nc.vector.tensor_mul(ost[:], t2[:], recip[:])
                nc.sync.dma_start(ost[127:128, :], degj_raw[:])
                nc.sync.dma_start(youts[bi][:], ost[:])

    nc.compile()
    return nc


def kernel(x, W, b, idx_i, idx_j):
    in_maps, ni_a, ni_c = prep_inputs(x, W, b, idx_i, idx_j)
    nc = build_program(ni_a, ni_c)
    res = run_bass_kernel_spmd(nc, in_maps, list(range(NCORES)))
    y = np.empty((B, O, N), np.float32)
    for s in range(NCORES):
        for bi in range(B):
            y[bi, :, s * SLICE:(s + 1) * SLICE] = res.results[s][f"y{bi}"]
    return y


if __name__ == "__main__":
    rng = np.random.default_rng(0)
    x = rng.standard_normal((B, C, N), np.float32)
    W = rng.standard_normal((127, C, 2), np.float32) * 0.05
    b = rng.standard_normal((127,), np.float32) * 0.05
    idx_i = rng.integers(0, N, 131072)
    idx_j = rng.integers(0, N, 131072)
    y = kernel(x, W, b, idx_i, idx_j)
    print("ok", y.shape, float(np.abs(y).mean()))


# revision 17
# speedup vs baseline: 1.1255x; 1.0175x over previous
"""Trainium2 Bass kernel for nn_PairwiseConv (gnn_message_passing).

Reference computation, for each edge e=(i,j) of a sparse adjacency:
    pair[b,o,e] = sum_c W[o,c,0]*x[b,c,i] + W[o,c,1]*x[b,c,j] + bias[o]
    y[b,o,n]    = (sum_{e: i_e=n} pair[b,o,e]) / max(deg_j[n],1)
    y[b,127,n]  = deg_j[n]            (counts channel)
where deg_j[n] = #{e: j_e = n}.

Algebraic reformulation used here (exact):
    y[b,o,n] = (deg_i[n]*(W0x[b,o,n] + bias[o]) + S[b,o,n]) / max(deg_j[n],1)
    S[b,o,n] = sum_m z[b,o,m] * AT[m,n],   z = W1^T x   (plus an all-ones
               row o=127 so that S[b,127,n] = deg_i[n])
    AT[m,n]  = #{e: j_e = m, i_e = n}  (edge-count matrix)
so the irregular gather/scatter becomes one dense [128,4096]x[4096,512]
matmul per (batch, node-slice) against the on-device-built count matrix.

Sharding: 8 cores = 8 slices of 512 output nodes; each core computes all 4
batches for its slice. AT[:, slice] is built on device from host-packed
per-partition (index,count) tables via GPSIMD local_scatter (32 tiles of
[128 rows, 512 cols], one per 128-row chunk of the source-node axis).
deg_j and deg_i are built the same way into [128,512] count matrices
(edges spread round-robin over the 128 partitions) and reduced with
all-ones matmuls, which also broadcast the degrees to all 128 partitions.

Host-side work is limited to formatting: slicing/deduplicating edge lists
into padded scatter tables, rotating x so every core sees its slice at
column 0 (keeps the SPMD program identical across cores), and
concatenating the 8 output tiles.
"""

import numpy as np
import ml_dtypes

import concourse.bass as bass
import concourse.mybir as mybir
import concourse.tile as tile
from concourse import bacc
from concourse.bass_utils import run_bass_kernel_spmd

B = 4
C = 128  # in channels
O = 128  # out channels incl. counts row (127 real + ones row)
N = 4096
SLICE = 512  # output nodes per core
NCORES = 8
MC = N // 128  # 32 source-node chunks
F32 = mybir.dt.float32
BF16 = mybir.dt.bfloat16
I16 = mybir.dt.int16
BF16_NP = ml_dtypes.bfloat16


def _pack_tables(rows, cols, nrows, ncols, ni=None):
    """Group (row, col) pairs by partition p=row%128 (and chunk row//128),
    dedup, and pack into [128, nchunk*NI] int16 index / bf16 count tables.

    rows in [0, nrows), cols in [0, ncols). Returns (idx, val, NI).
    """
    nchunk = nrows // 128
    key = rows * ncols + cols
    uniq, counts = np.unique(key, return_counts=True)
    ur = uniq // ncols
    uc = uniq % ncols
    chunk = ur // 128
    p = ur % 128
    # sort by (chunk, p) to get per-(chunk,p) runs
    order = np.lexsort((uc, p, chunk))
    chunk, p, uc, counts = chunk[order], p[order], uc[order], counts[order]
    gid = chunk * 128 + p
    # per (chunk,p) counts
    percell = np.bincount(gid, minlength=nchunk * 128)
    ni = ni if ni is not None else int(percell.max())
    ni += ni % 2  # even
    ni = max(ni, 2)
    idx = np.full((nchunk * 128, ni), -1, np.int16)
    val = np.zeros((nchunk * 128, ni), BF16_NP)
    pos = np.arange(len(gid)) - np.concatenate(([0], np.cumsum(percell)))[gid]
    idx[gid, pos] = uc.astype(np.int16)
    val[gid, pos] = counts.astype(BF16_NP)
    # [nchunk*128, ni] -> [128, nchunk*ni]
    idx = idx.reshape(nchunk, 128, ni).transpose(1, 0, 2).reshape(128, nchunk * ni)
    val = val.reshape(nchunk, 128, ni).transpose(1, 0, 2).reshape(128, nchunk * ni)
    return np.ascontiguousarray(idx), np.ascontiguousarray(val), ni


def prep_inputs(x, W, b, idx_i, idx_j):
    """Returns (in_maps, NI_A, NI_C): per-core input dicts + table widths."""
    x = np.ascontiguousarray(np.asarray(x, np.float32))
    W = np.asarray(W, np.float32)
    bias = np.asarray(b, np.float32)
    ii = np.asarray(idx_i).astype(np.int64)
    jj = np.asarray(idx_j).astype(np.int64)

    # weights: lhsT layouts [K=c, M=o], padded to 128 with a zero column
    W0T = np.zeros((128, 128), BF16_NP)
    W0T[:, :127] = W[:, :, 0].T.astype(BF16_NP)
    W1T = np.zeros((128, 128), BF16_NP)
    W1T[:, :127] = W[:, :, 1].T.astype(BF16_NP)
    bcol = np.zeros((128, 1), np.float32)
    bcol[:127, 0] = bias

    # first pass: compute per-core tables, track global max widths
    perc = []
    for s in range(NCORES):
        base = s * SLICE
        # AT build: edges with destination i in slice; row = rotated source
        sel = (ii >= base) & (ii < base + SLICE)
        m_rot = (jj[sel] - base) % N
        icol = ii[sel] - base
        # pack chunk-pairs: row' in [0, N/2), col' in [0, 1024)
        a_rows = (m_rot // 256) * 128 + (m_rot % 128)
        a_cols = icol + SLICE * ((m_rot // 128) % 2)
        # deg_j build: edges with j in slice, spread over partitions
        selj = (jj >= base) & (jj < base + SLICE)
        nj = int(selj.sum())
        c_rows = np.arange(nj, dtype=np.int64) % 128
        c_cols = jj[selj] - base
        # deg_i build: the i-filtered edge set, spread over partitions
        ni_ = int(sel.sum())
        d_rows = np.arange(ni_, dtype=np.int64) % 128
        d_cols = icol
        perc.append((a_rows, a_cols, c_rows, c_cols, d_rows, d_cols))

    # uniform NI across cores (SPMD program shapes must match)
    ni_a = ni_c = 0
    for a_rows, a_cols, c_rows, c_cols, d_rows, d_cols in perc:
        _, _, na = _pack_tables(a_rows, a_cols, N // 2, 2 * SLICE)
        _, _, nc_ = _pack_tables(c_rows, c_cols, 128, SLICE)
        _, _, nd = _pack_tables(d_rows, d_cols, 128, SLICE)
        ni_a, ni_c = max(ni_a, na), max(ni_c, max(nc_, nd))

    in_maps = []
    for s in range(NCORES):
        a_rows, a_cols, c_rows, c_cols, d_rows, d_cols = perc[s]
        idxA, valA, _ = _pack_tables(a_rows, a_cols, N // 2, 2 * SLICE, ni=ni_a)
        idxC, valC, _ = _pack_tables(c_rows, c_cols, 128, SLICE, ni=ni_c)
        idxD, valD, _ = _pack_tables(d_rows, d_cols, 128, SLICE, ni=ni_c)
        m = {
            "W0T": W0T,
            "W1T": W1T,
            "bcol": bcol,
            "idxA": idxA,
            "valA": valA,
            "idxC": np.ascontiguousarray(np.concatenate([idxC, idxD], axis=1)),
            "valC": np.ascontiguousarray(np.concatenate([valC, valD], axis=1)),
        }
        for bi in range(B):
            m[f"x{bi}"] = np.ascontiguousarray(
                np.roll(x[bi], -s * SLICE, axis=1).astype(BF16_NP))
        in_maps.append(m)
    return in_maps, ni_a, ni_c


def build_program(ni_a, ni_c):
    nc = bacc.Bacc("TRN2", target_bir_lowering=False, debug=False, num_devices=NCORES)

    xs = [nc.dram_tensor(f"x{bi}", [C, N], BF16, kind="ExternalInput") for bi in range(B)]
    W0T = nc.dram_tensor("W0T", [128, 128], BF16, kind="ExternalInput")
    W1T = nc.dram_tensor("W1T", [128, 128], BF16, kind="ExternalInput")
    bcol = nc.dram_tensor("bcol", [128, 1], F32, kind="ExternalInput")
    idxA = nc.dram_tensor("idxA", [128, (MC // 2) * ni_a], I16, kind="ExternalInput")
    valA = nc.dram_tensor("valA", [128, (MC // 2) * ni_a], BF16, kind="ExternalInput")
    idxC = nc.dram_tensor("idxC", [128, 2 * ni_c], I16, kind="ExternalInput")
    valC = nc.dram_tensor("valC", [128, 2 * ni_c], BF16, kind="ExternalInput")
    youts = [nc.dram_tensor(f"y{bi}", [O, SLICE], F32, kind="ExternalOutput")
             for bi in range(B)]

    with tile.TileContext(nc) as tc:
        with (
            tc.tile_pool(name="const", bufs=1) as constp,
            tc.tile_pool(name="scat", bufs=1) as scatp,
            tc.tile_pool(name="at", bufs=1) as atp,
            tc.tile_pool(name="xp", bufs=1) as xp,
            tc.tile_pool(name="zt", bufs=1) as ztp,
            tc.tile_pool(name="work", bufs=1) as workp,
            tc.tile_pool(name="small", bufs=4) as smallp,
            tc.tile_pool(name="ps_zt", bufs=2, space="PSUM") as ps_zt,
            tc.tile_pool(name="ps_s", bufs=1, space="PSUM") as ps_s,
            tc.tile_pool(name="ps_deg", bufs=1, space="PSUM") as ps_deg_p,
            tc.tile_pool(name="ps_di", bufs=1, space="PSUM") as ps_di_p,
        ):
            # ---- loads: x on sync+scalar HWDGE queues, tables on gpsimd ----
            w1t = constp.tile([128, 128], BF16)
            nc.sync.dma_start(w1t[:], W1T[:])
            half = N // 2
            xbs = []
            for bi in range(B):
                xb = xp.tile([C, N], BF16, tag=f"xb{bi}", name=f"xb{bi}")
                eng = nc.sync if bi < 2 else nc.scalar
                if bi == 0:
                    q = N // 4
                    for qi in range(4):
                        eng.dma_start(xb[:, qi * q:(qi + 1) * q],
                                      xs[bi][:, qi * q:(qi + 1) * q])
                else:
                    eng.dma_start(xb[:, :half], xs[bi][:, :half])
                    eng.dma_start(xb[:, half:], xs[bi][:, half:])
                xbs.append(xb)
            iC = scatp.tile([128, 2 * ni_c], I16)
            nc.gpsimd.dma_start(iC[:], idxC[:])
            vC = scatp.tile([128, 2 * ni_c], BF16)
            nc.gpsimd.dma_start(vC[:], valC[:])
            iA = scatp.tile([128, (MC // 2) * ni_a], I16)
            nc.gpsimd.dma_start(iA[:], idxA[:])
            vA = scatp.tile([128, (MC // 2) * ni_a], BF16)
            nc.gpsimd.dma_start(vA[:], valA[:])
            w0t = constp.tile([128, 128], BF16)
            nc.scalar.dma_start(w0t[:], W0T[:])
            bc = constp.tile([128, 1], F32)
            nc.scalar.dma_start(bc[:], bcol[:])
            ones128 = constp.tile([128, 128], BF16)
            nc.vector.memset(ones128[:], 1.0)

            # ---- count-matrix scatters (GPSIMD): deg_j, deg_i, then AT ----
            cC = constp.tile([128, SLICE], BF16)
            nc.gpsimd.local_scatter(
                out_ap=cC[:], data_ap=vC[:, :ni_c], idxs_ap=iC[:, :ni_c],
                channels=128, num_elems=SLICE, num_idxs=ni_c,
            )
            cI = constp.tile([128, SLICE], BF16)
            nc.gpsimd.local_scatter(
                out_ap=cI[:], data_ap=vC[:, ni_c:], idxs_ap=iC[:, ni_c:],
                channels=128, num_elems=SLICE, num_idxs=ni_c,
            )
            at = atp.tile([128, MC * SLICE], BF16)
            for k in range(MC // 2):
                nc.gpsimd.local_scatter(
                    out_ap=at[:, k * 1024:(k + 1) * 1024],
                    data_ap=vA[:, k * ni_a:(k + 1) * ni_a],
                    idxs_ap=iA[:, k * ni_a:(k + 1) * ni_a],
                    channels=128, num_elems=1024, num_idxs=ni_a,
                )

            # ---- phase A: zT builds for all batches ----
            zts = []
            for bi in range(B):
                xb = xbs[bi]
                zt = ztp.tile([128, N], BF16, tag=f"zt{bi}", name=f"zt{bi}")
                for g in range(MC // 4):  # 8 psum groups of 4 chunks
                    pz = ps_zt.tile([128, 512], F32, tag="pz", name=f"pz{bi}_{g}")
                    for kk in range(4):
                        mc = g * 4 + kk
                        nc.tensor.matmul(
                            pz[:, kk * 128:(kk + 1) * 128],
                            xb[:, mc * 128:(mc + 1) * 128],
                            w1t[:],
                            start=True, stop=True,
                        )
                    if g % 2 == 0:
                        nc.vector.tensor_copy(zt[:, g * 512:(g + 1) * 512], pz[:])
                    else:
                        nc.scalar.copy(zt[:, g * 512:(g + 1) * 512], pz[:])
                zts.append(zt)

            # ---- degree reductions (batch-independent, all-ones matmuls) ----
            ps_deg = ps_deg_p.tile([128, SLICE], F32)
            nc.tensor.matmul(ps_deg[:], ones128[:], cC[:], start=True, stop=True)
            ps_di = ps_di_p.tile([128, SLICE], F32)
            nc.tensor.matmul(ps_di[:], ones128[:], cI[:], start=True, stop=True)
            degj_raw = smallp.tile([1, SLICE], F32)
            nc.scalar.copy(degj_raw[:], ps_deg[0:1, :])
            rmax = workp.tile([128, SLICE], F32)
            nc.vector.tensor_scalar_max(rmax[:], ps_deg[:], 1.0)
            recip = workp.tile([128, SLICE], F32)
            nc.vector.reciprocal(recip[:], rmax[:])

            # ---- u_b = W0^T x_b(slice); t1 = (u + bias) * deg_i ----
            t1s = []
            for bi in range(B):
                ps_u = ps_zt.tile([128, 512], F32, tag="pz", name=f"ps_u{bi}")
                nc.tensor.matmul(ps_u[:], w0t[:], xbs[bi][:, :SLICE],
                                 start=True, stop=True)
                ub = smallp.tile([128, SLICE], F32, tag="ub", name=f"ub{bi}")
                nc.vector.tensor_scalar_add(ub[:], ps_u[:], bc[:, :1])
                t1 = smallp.tile([128, SLICE], F32, tag=f"t1{bi}", name=f"t1{bi}")
                nc.vector.tensor_mul(t1[:], ub[:], ps_di[:])
                t1s.append(t1)

            # ---- phase B: big matmuls, chunk-major; tail batch-major ----
            TAIL = 4
            ps_Ss = [ps_s.tile([128, SLICE], F32, tag=f"ps{bi}", name=f"ps_S{bi}")
                     for bi in range(B)]
            for mc in range(MC - TAIL):
                for bi in range(B):
                    nc.tensor.matmul(
                        ps_Ss[bi][:],
                        zts[bi][:, mc * 128:(mc + 1) * 128],
                        at[:, mc * SLICE:(mc + 1) * SLICE],
                        start=(mc == 0), stop=False,
                        skip_group_check=True,
                    )
            for bi in range(B):
                for mc in range(MC - TAIL, MC):
                    nc.tensor.matmul(
                        ps_Ss[bi][:],
                        zts[bi][:, mc * 128:(mc + 1) * 128],
                        at[:, mc * SLICE:(mc + 1) * SLICE],
                        start=False, stop=(mc == MC - 1),
                        skip_group_check=True,
                    )
                t2 = smallp.tile([128, SLICE], F32, tag="t2", name=f"t2{bi}")
                nc.vector.tensor_add(t2[:], t1s[bi][:], ps_Ss[bi][:])
                ost = workp.tile([O, SLICE], F32, tag=f"ost{bi}", name=f"ost{bi}")
                nc.vector.tensor_mul(ost[:], t2[:], recip[:])
                nc.sync.dma_start(ost[127:128, :], degj_raw[:])
                nc.sync.dma_start(youts[bi][:], ost[:])

    nc.compile()
    return nc


def kernel(x, W, b, idx_i, idx_j):
    in_maps, ni_a, ni_c = prep_inputs(x, W, b, idx_i, idx_j)
    nc = build_program(ni_a, ni_c)
    res = run_bass_kernel_spmd(nc, in_maps, list(range(NCORES)))
    y = np.empty((B, O, N), np.float32)
    for s in range(NCORES):
        for bi in range(B):
            y[bi, :, s * SLICE:(s + 1) * SLICE] = res.results[s][f"y{bi}"]
    return y


if __name__ == "__main__":
    rng = np.random.default_rng(0)
    x = rng.standard_normal((B, C, N), np.float32)
    W = rng.standard_normal((127, C, 2), np.float32) * 0.05
    b = rng.standard_normal((127,), np.float32) * 0.05
    idx_i = rng.integers(0, N, 131072)
    idx_j = rng.integers(0, N, 131072)
    y = kernel(x, W, b, idx_i, idx_j)
    print("ok", y.shape, float(np.abs(y).mean()))
